# revision 37
# baseline (speedup 1.0000x reference)
"""DCT heat-blur kernel for Trainium2 (8 NeuronCores, Bass/Tile).

Math (matches reference einsums): per image X,
    out = (D diag(e) D) X (D diag(e) D)^T = W^T X W,  W = (D diag(e) D)^T,
with e[k] = exp(-(pi k/N)^2 sigma_b^2/2).  W_b is built on host per batch.
Device: t1 = X^T W, out = t1^T W  (two GEMM passes per image).

Hybrid precision (validated by exact host emulation of the quantization
on the real inputs; harness tolerance is 2e-2 max-rel, achieved ~1e-2):
  class 0 (k<=1):    X f16,  W f16,  f16 drains            (err ~8e-4)
  class 1 (2<=k<=5): X f8e4, W bf16, bf16-punned drains    (err <=1.1e-2)
  class 2 (k>=6):    X f8e4, W f8e4, t1 f8, bf16-pun out   (err <=9.7e-3)
Class-2 GEMMs use fp8 DoubleRow matmuls: both K=128 k-tiles of the K=256
contraction in ONE instruction at 2x MAC rate (~107ns per [128x256]
output vs 2x107 for fp16 accumulation pairs; fp8 with PSUM-accumulation
pairs falls back to 1x, so DoubleRow is required for the 2x).  fp8 X/W
also halve input DMA.  Blur washes out quantization noise, so larger
sigma tolerates coarser inputs.

PSUM drains (psum f32 -> sbuf) are the kernel bottleneck: only the
vector and scalar engines can read PSUM (GPSIMD cannot, DMA cannot),
at ~1 element/lane/cycle regardless of dtype => ~590-690ns per
[128, 512-elem] drain.  Two drains per image (t1, out) over two engines
=> ~590ns/image floor.  The "bf16 pun" (bitcast the f32 psum to bf16
pairs, copy the high halves = round-toward-zero bf16) keeps the drains
single-pass while producing 16-bit operands for the second GEMM and the
output store.

Schedule per core (48 images):
 - classes INTERLEAVED (a class0/1 image every ~4 positions, first 4
   positions class0): f16 images are PE-bound (872ns) while f8 images
   are drain-bound (~600ns), so interleaving overlaps their idle times.
 - depth-3 software pipeline: t2 of position p is emitted after t1 of
   position p+3, so every drain has a deep backlog and semaphore
   latency never idles the copy engines.
 - all transfers on the sync HW-DGE ring (gpsimd software-DGE transfers
   are several times slower); partition-major DRAM layouts give >=1KB
   contiguous runs per descriptor; loads dispatched in need-order.
 - a short vector-memset + dummy-matmul bridge covers engine boot and
   the ~4us DGE ring spin-up before the first input lands.
 - per-slot grouped stores; per-image near the end to shorten the tail.

Sharding: pure data parallel over batches; batches sorted by (class, k)
and dealt 8-way so every core gets identical per-class slot counts
(a,b,c) -- required because all cores run one SPMD NEFF.  The NEFF is
compiled on first call for the (a,b,c) derived from the actual
fwd_steps (cached per (a,b,c)).
"""

import os
import numpy as np
import ml_dtypes

BATCH = 128
CHANNELS = 3
N = 256
N_CORES = 8
PB = BATCH // N_CORES          # batch slots per core
IMGS = PB * CHANNELS           # images per core
NWARM = 15

F8 = ml_dtypes.float8_e4m3
BF16 = ml_dtypes.bfloat16

LAST_EXEC_TIME_NS = None
_NC_CACHE = {}


def _install_ntff_hook():
    """Wire antenv.axon_hooks (missing in this image) so trace=True works."""
    import sys
    import types

    if "antenv.axon_hooks" in sys.modules:
        return
    try:
        import trn_agent_boot.trn_boot as tb

        hook = tb._ntff_profile_via_ctypes("/opt/axon/libaxon_pjrt.so")
    except Exception:
        hook = None
    m = types.ModuleType("antenv.axon_hooks")
    m.get_axon_ntff_profile_hook = lambda: hook
    m.set_axon_ntff_profile_hook = lambda h: None
    sys.modules["antenv.axon_hooks"] = m


def _build_nc(a, b, c):
    import concourse.bacc as bacc
    import concourse.tile as tile
    import concourse.mybir as mybir

    f32 = mybir.dt.float32
    f16 = mybir.dt.float16
    bf = mybir.dt.bfloat16
    f8 = mybir.dt.float8e4

    nF = 3 * a                  # f16 images (class 0)
    n8 = 3 * (b + c)            # f8 images (classes 1, 2)
    ab = a + b

    nc = bacc.Bacc("TRN2", target_bir_lowering=False, debug=False)
    xF_d = x8_d = wF_d = wM_d = w8_d = oF_d = o8_d = None
    if nF:
        xF_d = nc.dram_tensor("xF", [128, nF, 2, N], f16, kind="ExternalInput").ap()
        oF_d = nc.dram_tensor("oF", [128, nF, 2, N], f16, kind="ExternalOutput").ap()
        wF_d = nc.dram_tensor("wF", [128, a, 2, N], f16, kind="ExternalInput").ap()
    if n8:
        x8_d = nc.dram_tensor("x8", [128, n8, 2, N], f8, kind="ExternalInput").ap()
        o8_d = nc.dram_tensor("o8", [128, n8, 2, N], bf, kind="ExternalOutput").ap()
    if b:
        wM_d = nc.dram_tensor("wM", [128, b, 2, N], bf, kind="ExternalInput").ap()
    if c:
        w8_d = nc.dram_tensor("w8", [128, c, 2, N], f8, kind="ExternalInput").ap()

    # interleave classes: a class0/1 image every ~IMGS/n01 positions so the
    # PE-bound f16 images and drain-bound f8 images overlap their idle times
    n01 = 3 * ab
    head = min(4, n01)
    seq = list(range(head))
    i01, i2, err = head, 0, 0
    rem01 = n01 - head
    rem = IMGS - head
    for p in range(rem):
        err += rem01
        if err >= rem and i01 < n01:
            err -= rem
            seq.append(i01)
            i01 += 1
        elif i2 < IMGS - n01:
            seq.append(n01 + i2)
            i2 += 1
        else:
            seq.append(i01)
            i01 += 1
    pos_of = {img: p for p, img in enumerate(seq)}

    # x8 load chunks (front-loaded small), w8 chunks
    x8_cuts = [0]
    for step in (3, 6, 9, 9, 9, 9, 9):
        if x8_cuts[-1] >= n8:
            break
        x8_cuts.append(min(x8_cuts[-1] + step, n8))
    x8_chunks = list(zip(x8_cuts, x8_cuts[1:]))
    w8_cuts = [0]
    for step in (2, 5, 5, 5):
        if w8_cuts[-1] >= c:
            break
        w8_cuts.append(min(w8_cuts[-1] + step, c))
    w8_chunks = list(zip(w8_cuts, w8_cuts[1:]))

    with tile.TileContext(nc) as tc:
        with (
            tc.tile_pool(name="const", bufs=1) as cpool,
            tc.tile_pool(name="xpool", bufs=1) as xpool,
            tc.tile_pool(name="tpF", bufs=5) as tpF,
            tc.tile_pool(name="tpB", bufs=5) as tpB,
            tc.tile_pool(name="tp8", bufs=5) as tp8,
            tc.tile_pool(name="opool", bufs=5) as opool,
            tc.tile_pool(name="ps1", bufs=4, space="PSUM") as ps1,
            tc.tile_pool(name="ps2", bufs=4, space="PSUM") as ps2,
        ):
            # PE warmup bridge (vector engine boots fast): keeps the PE
            # busy while the first input transfers land (~5us)
            wu_sb = cpool.tile([128, 128], f16, name="wu")
            nc.vector.memset(wu_sb[:], 0.25)
            for _ in range(NWARM):
                wu_ps = ps2.tile([128, 2, N], f32, tag="t2")
                nc.tensor.matmul(
                    wu_ps[:, 0, 0:128], lhsT=wu_sb[:], rhs=wu_sb[:],
                    start=True, stop=True,
                )

            # loads, need-order, all on the sync HW-DGE ring (gpsimd
            # software-DGE transfers are several times slower)
            wF_sb = wM_sb = w8_sb = xF_sb = x8_sb = None
            if a:
                wF_sb = cpool.tile([128, a, 2, N], f16, name="wF")
            if nF:
                xF_sb = xpool.tile([128, nF, 2, N], f16, name="xF")
            if b:
                wM_sb = cpool.tile([128, b, 2, N], bf, name="wM")
            if n8:
                x8_sb = xpool.tile([128, n8, 2, N], f8, name="x8")
            if c:
                w8_sb = cpool.tile([128, c, 2, N], f8, name="w8")

            def load_x8(i):
                if i < len(x8_chunks):
                    s, e = x8_chunks[i]
                    nc.sync.dma_start(x8_sb[:, s:e], x8_d[:, s:e])

            def load_w8(i):
                if i < len(w8_chunks):
                    s, e = w8_chunks[i]
                    nc.sync.dma_start(w8_sb[:, s:e], w8_d[:, s:e])

            def load_xF(s, e):
                if nF and e > min(s, nF):
                    e = min(e, nF)
                    nc.sync.dma_start(xF_sb[:, s:e], xF_d[:, s:e])

            if a:
                nc.sync.dma_start(wF_sb[:], wF_d)
            load_xF(0, 2)
            load_w8(0)
            load_x8(0)
            load_xF(2, 4)
            load_x8(1)
            if b:
                nc.sync.dma_start(wM_sb[:], wM_d)
            load_w8(1)
            load_x8(2)
            load_xF(4, nF)
            load_w8(2)
            for i in range(3, len(x8_chunks)):
                load_x8(i)
            load_w8(3)

            def img_ctx(img):
                slot = img // 3
                cls = 0 if slot < a else (1 if slot < ab else 2)
                if cls == 0:
                    xt, wv = xF_sb[:, img], wF_sb[:, slot]
                elif cls == 1:
                    xt, wv = x8_sb[:, img - nF], wM_sb[:, slot - a]
                else:
                    xt, wv = x8_sb[:, img - nF], w8_sb[:, slot - ab]
                return slot, cls, xt, wv

            def copy(out_ap, ps_ap, cls, eng):
                if cls != 0:
                    b16 = ps_ap.bitcast(bf)
                    in_ap = b16[:, :, 1::2] if len(b16.shape) == 3 else b16[:, 1::2]
                else:
                    in_ap = ps_ap
                if eng == 0:
                    nc.vector.tensor_copy(out=out_ap, in_=in_ap)
                else:
                    nc.scalar.copy(out_ap, in_ap)

            t1sb = {}
            otiles = {}

            def stage_t1(img, pos):
                slot, cls, xt, wv = img_ctx(img)
                t1_ps = ps1.tile([128, 2, N], f32)
                for mb in range(2):
                    if cls == 2:
                        # fp8 DoubleRow: both k-tiles in one instruction at
                        # 2x MAC rate (accumulating fp8 pairs fall to 1x)
                        nc.tensor.matmul(
                            t1_ps[:, mb, :],
                            lhsT=xt[:, :, mb * 128:(mb + 1) * 128],
                            rhs=wv[:], start=True, stop=True,
                            perf_mode=mybir.MatmulPerfMode.DoubleRow,
                        )
                        continue
                    for aa in range(2):
                        nc.tensor.matmul(
                            t1_ps[:, mb, :],
                            lhsT=xt[:, aa, mb * 128:(mb + 1) * 128],
                            rhs=wv[:, aa],
                            start=(aa == 0),
                            stop=(aa == 1),
                        )
                if cls == 0:
                    t1_sb = tpF.tile([128, 2, N], f16)
                elif cls == 1:
                    t1_sb = tpB.tile([128, 2, N], bf)
                else:
                    t1_sb = tp8.tile([128, 2, N], f8)
                copy(t1_sb[:], t1_ps[:], cls, eng=pos % 2)
                t1sb[img] = t1_sb

            def stage_t2(img, pos):
                slot, cls, xt, wv = img_ctx(img)
                t1_sb = t1sb.pop(img)
                t2_ps = ps2.tile([128, 2, N], f32, tag="t2")
                for mb in range(2):
                    if cls == 2:
                        nc.tensor.matmul(
                            t2_ps[:, mb, :],
                            lhsT=t1_sb[:, :, mb * 128:(mb + 1) * 128],
                            rhs=wv[:], start=True, stop=True,
                            perf_mode=mybir.MatmulPerfMode.DoubleRow,
                        )
                        continue
                    for aa in range(2):
                        nc.tensor.matmul(
                            t2_ps[:, mb, :],
                            lhsT=t1_sb[:, aa, mb * 128:(mb + 1) * 128],
                            rhs=wv[:, aa],
                            start=(aa == 0),
                            stop=(aa == 1),
                        )
                j = img % 3
                if j == 0:
                    otiles[slot] = opool.tile(
                        [128, 3, 2, N], f16 if cls == 0 else bf,
                        tag=f"o{min(cls, 1)}", name=f"ot{min(cls, 1)}",
                    )
                ot = otiles[slot]
                last = pos >= IMGS - 4
                if last:
                    copy(ot[:, j, 0], t2_ps[:, 0], cls, eng=0)
                    copy(ot[:, j, 1], t2_ps[:, 1], cls, eng=1)
                else:
                    copy(ot[:, j], t2_ps[:], cls, eng=(pos + 1) % 2)

                od = oF_d if cls == 0 else o8_d
                base = 0 if cls == 0 else nF
                odi = od[:, 3 * slot + j - base]
                # tail-mode slot: its last image lands in the final positions,
                # so store per-image to keep the drain-out overlapped
                tail = pos_of[3 * slot + 2] >= IMGS - 6
                if last or tail:
                    # alternate HW-DGE queues so tail dispatches overlap
                    q = nc.sync if pos % 2 == 0 else nc.scalar
                    q.dma_start(odi, ot[:, j])
                elif j == 2:
                    nc.sync.dma_start(
                        od[:, 3 * slot - base:3 * slot + 3 - base], ot[:])

            # depth-3 software pipeline over the interleaved order: t2 of
            # position p sits after t1 of position p+3, giving every drain
            # a deep backlog so semaphore latency never idles the engines
            D = 3
            for p in range(IMGS):
                stage_t1(seq[p], p)
                if p >= D:
                    stage_t2(seq[p - D], p - D)
            for p in range(IMGS - D, IMGS):
                stage_t2(seq[p], p)

    nc.compile()
    return nc


def _get_nc(a, b, c):
    key = (a, b, c)
    if key not in _NC_CACHE:
        _NC_CACHE[key] = _build_nc(a, b, c)
    return _NC_CACHE[key]


def _host_w(blur_sigmas, fwd_steps):
    """Per-batch W_b = (D diag(e_b) D)^T, float64, [B, N, N]."""
    sig = np.asarray(blur_sigmas, dtype=np.float64)
    steps = np.asarray(fwd_steps).astype(np.int64)
    n = np.arange(N, dtype=np.float64)
    D = np.sqrt(2.0 / N) * np.cos(np.pi * (n[None, :] + 0.5) * n[:, None] / N)
    D[0] *= 1.0 / np.sqrt(2.0)
    freqs = np.pi * n / N
    uniq, inv = np.unique(steps, return_inverse=True)
    ms = np.empty((len(uniq), N, N), dtype=np.float64)
    for i, s in enumerate(uniq):
        t = sig[s] ** 2 / 2.0
        e = np.exp(-(freqs ** 2) * t)
        ms[i] = (D @ (e[:, None] * D)).T
    return ms[inv]


def _classify(steps):
    """class 0: k<=1 (f16), 1: 2<=k<=5 (mixed), 2: k>=6 (f8)."""
    k = np.asarray(steps).astype(np.int64)
    return np.where(k <= 1, 0, np.where(k <= 5, 1, 2))


def kernel(x, blur_sigmas, fwd_steps):
    global LAST_EXEC_TIME_NS
    from concourse import bass_utils

    x = np.asarray(x)
    assert x.shape == (BATCH, CHANNELS, N, N), x.shape
    steps = np.asarray(fwd_steps).astype(np.int64)
    cls = _classify(steps)
    n0 = int((cls == 0).sum())
    n1 = int((cls == 1).sum())
    a = min(-(-n0 // N_CORES), PB)
    ab = min(max(a, -(-(n0 + n1) // N_CORES)), PB)
    b = ab - a
    c = PB - ab

    # batch order: sort by (class, k); slot s of core i is batch order[8s+i]
    order = np.lexsort((steps, cls))
    w_all = _host_w(blur_sigmas, fwd_steps)

    # device image layout [128, 2, N]: [p, aa, w] = X[aa*128+p, w]
    xr = x.astype(np.float32).reshape(BATCH, CHANNELS, 2, 128, N)

    in_maps = []
    for core in range(N_CORES):
        bs = order[core::N_CORES]
        assert len(bs) == PB
        m = {}
        if a:
            xf = xr[bs[:a]].transpose(3, 0, 1, 2, 4).reshape(128, 3 * a, 2 * N)
            m["xF"] = np.ascontiguousarray(xf.astype(np.float16))
        if b + c:
            x8 = xr[bs[a:]].transpose(3, 0, 1, 2, 4).reshape(128, 3 * (b + c), 2 * N)
            m["x8"] = np.ascontiguousarray(x8.astype(F8))
        # W layout [128, slot, aa, h] = W_b[aa*128+p, h]
        wb = w_all[bs].reshape(PB, 2, 128, N).transpose(2, 0, 1, 3)
        if a:
            m["wF"] = np.ascontiguousarray(wb[:, :a].astype(np.float16))
        if b:
            m["wM"] = np.ascontiguousarray(wb[:, a:ab].astype(BF16))
        if c:
            m["w8"] = np.ascontiguousarray(wb[:, ab:].astype(F8))
        in_maps.append(m)

    nc = _get_nc(a, b, c)
    trace = os.environ.get("BASS_DCT_TRACE", "0") == "1"
    if trace:
        _install_ntff_hook()

    res = None
    exec_ns = None
    # first execution of a freshly loaded NEFF is occasionally slow (cold
    # device caches) or raises a transient NRT error; run once untraced to
    # warm up, then measure the steady-state execution with trace
    import tempfile
    for attempt in range(4):
        try:
            kwargs = {}
            if trace and attempt > 0:
                kwargs["trace"] = True
                base = os.environ.get("BASS_DCT_TRACE_DIR")
                if base:
                    td = os.path.join(base, f"attempt{attempt}")
                    os.makedirs(td, exist_ok=True)
                else:
                    td = tempfile.mkdtemp(prefix="dct_trace_")
                kwargs["tmpdir"] = td
            r = bass_utils.run_bass_kernel_spmd(
                nc, in_maps, core_ids=list(range(N_CORES)), **kwargs
            )
            res = r
            if r.exec_time_ns is not None:
                exec_ns = r.exec_time_ns
            if attempt >= 1:
                break
        except Exception:
            if attempt == 3 and res is None:
                raise
            import time as _time

            _time.sleep(2.0)
    LAST_EXEC_TIME_NS = exec_ns

    # gather: [img, p, (aa, w)] -> out[batch, ch, aa*128+p, w]
    out = np.empty((BATCH, CHANNELS, N, N), dtype=np.float32)
    for core in range(N_CORES):
        bs = order[core::N_CORES]
        parts = []
        if a:
            ocF = np.asarray(res.results[core]["oF"], dtype=np.float32)
            parts.append(ocF.reshape(128, 3 * a, 2, N))
        if b + c:
            oc8 = np.asarray(res.results[core]["o8"]).astype(np.float32)
            parts.append(oc8.reshape(128, 3 * (b + c), 2, N))
        oc = np.concatenate(parts, axis=1) if len(parts) > 1 else parts[0]
        # [p, img, aa, w] -> [img, aa, p, w]
        oc = oc.transpose(1, 2, 0, 3).reshape(PB, CHANNELS, N, N)
        out[bs] = oc
    return np.ascontiguousarray(out)


# revision 38
# speedup vs baseline: 1.1403x; 1.1403x over previous
"""DCT heat-blur kernel for Trainium2 (8 NeuronCores, Bass/Tile).

Math (matches reference einsums): per image X,
    out = (D diag(e) D) X (D diag(e) D)^T = W^T X W,  W = (D diag(e) D)^T,
with e[k] = exp(-(pi k/N)^2 sigma_b^2/2).  W_b is built on host per batch.
Device: t1 = X^T W, out = t1^T W  (two GEMM passes per image).

Hybrid precision (validated by exact host emulation of the quantization
on the real inputs; harness tolerance is 2e-2 max-rel, achieved ~1e-2):
  class 0 (k<=1):    X f16,  W f16,  f16 drains            (err ~8e-4)
  class 1 (2<=k<=5): X f8e4, W bf16, bf16-punned drains    (err <=1.1e-2)
  class 2 (k>=6):    X f8e4, W f8e4, t1 f8, bf16-pun out   (err <=9.7e-3)
Class-2 GEMMs use fp8 DoubleRow matmuls: both K=128 k-tiles of the K=256
contraction in ONE instruction at 2x MAC rate (~107ns per [128x256]
output vs 2x107 for fp16 accumulation pairs; fp8 with PSUM-accumulation
pairs falls back to 1x, so DoubleRow is required for the 2x).  fp8 X/W
also halve input DMA.  Blur washes out quantization noise, so larger
sigma tolerates coarser inputs.

PSUM drains (psum f32 -> sbuf) are the kernel bottleneck: only the
vector and scalar engines can read PSUM (GPSIMD cannot, DMA cannot),
at ~1 element/lane/cycle regardless of dtype => ~590-690ns per
[128, 512-elem] drain.  Two drains per image (t1, out) over two engines
=> ~590ns/image floor.  The "bf16 pun" (bitcast the f32 psum to bf16
pairs, copy the high halves = round-toward-zero bf16) keeps the drains
single-pass while producing 16-bit operands for the second GEMM and the
output store.

Schedule per core (48 images):
 - classes INTERLEAVED (a class0/1 image every ~4 positions, first 4
   positions class0): f16 images are PE-bound (872ns) while f8 images
   are drain-bound (~600ns), so interleaving overlaps their idle times.
 - depth-3 software pipeline: t2 of position p is emitted after t1 of
   position p+3, so every drain has a deep backlog and semaphore
   latency never idles the copy engines.
 - all transfers on the sync HW-DGE ring (gpsimd software-DGE transfers
   are several times slower); partition-major DRAM layouts give >=1KB
   contiguous runs per descriptor; loads dispatched in need-order.
 - a short vector-memset + dummy-matmul bridge covers engine boot and
   the ~4us DGE ring spin-up before the first input lands.
 - per-slot grouped stores; per-image near the end to shorten the tail.

Sharding: pure data parallel over batches; batches sorted by (class, k)
and dealt 8-way so every core gets identical per-class slot counts
(a,b,c) -- required because all cores run one SPMD NEFF.  The NEFF is
compiled on first call for the (a,b,c) derived from the actual
fwd_steps (cached per (a,b,c)).
"""

import os
import numpy as np
import ml_dtypes

BATCH = 128
CHANNELS = 3
N = 256
N_CORES = 8
PB = BATCH // N_CORES          # batch slots per core
IMGS = PB * CHANNELS           # images per core
NWARM = 13

F8 = ml_dtypes.float8_e4m3
BF16 = ml_dtypes.bfloat16

LAST_EXEC_TIME_NS = None
_NC_CACHE = {}


def _install_ntff_hook():
    """Wire antenv.axon_hooks (missing in this image) so trace=True works."""
    import sys
    import types

    if "antenv.axon_hooks" in sys.modules:
        return
    try:
        import trn_agent_boot.trn_boot as tb

        hook = tb._ntff_profile_via_ctypes("/opt/axon/libaxon_pjrt.so")
    except Exception:
        hook = None
    m = types.ModuleType("antenv.axon_hooks")
    m.get_axon_ntff_profile_hook = lambda: hook
    m.set_axon_ntff_profile_hook = lambda h: None
    sys.modules["antenv.axon_hooks"] = m


def _build_nc(a, b, c):
    import concourse.bacc as bacc
    import concourse.tile as tile
    import concourse.mybir as mybir

    f32 = mybir.dt.float32
    f16 = mybir.dt.float16
    bf = mybir.dt.bfloat16
    f8 = mybir.dt.float8e4

    nF = 3 * a                  # f16 images (class 0)
    n8 = 3 * (b + c)            # f8 images (classes 1, 2)
    ab = a + b

    nc = bacc.Bacc("TRN2", target_bir_lowering=False, debug=False)
    xF_d = x8_d = wF_d = wM_d = w8_d = oF_d = o8_d = None
    if nF:
        xF_d = nc.dram_tensor("xF", [128, nF, 2, N], f16, kind="ExternalInput").ap()
        oF_d = nc.dram_tensor("oF", [128, nF, 2, N], f16, kind="ExternalOutput").ap()
        wF_d = nc.dram_tensor("wF", [128, a, 2, N], f16, kind="ExternalInput").ap()
    if n8:
        x8_d = nc.dram_tensor("x8", [128, n8, 2, N], f8, kind="ExternalInput").ap()
        o8_d = nc.dram_tensor("o8", [128, n8, 2, N], bf, kind="ExternalOutput").ap()
    if b:
        wM_d = nc.dram_tensor("wM", [128, b, 2, N], bf, kind="ExternalInput").ap()
    if c:
        w8_d = nc.dram_tensor("w8", [128, c, 2, N], f8, kind="ExternalInput").ap()

    # interleave classes: a class0/1 image every ~IMGS/n01 positions so the
    # PE-bound f16 images and drain-bound f8 images overlap their idle times
    n01 = 3 * ab
    head = min(4, n01)
    seq = list(range(head))
    i01, i2, err = head, 0, 0
    rem01 = n01 - head
    rem = IMGS - head
    for p in range(rem):
        err += rem01
        if err >= rem and i01 < n01:
            err -= rem
            seq.append(i01)
            i01 += 1
        elif i2 < IMGS - n01:
            seq.append(n01 + i2)
            i2 += 1
        else:
            seq.append(i01)
            i01 += 1
    pos_of = {img: p for p, img in enumerate(seq)}

    # x8 load chunks (front-loaded small), w8 chunks
    x8_cuts = [0]
    for step in (3, 6, 9, 9, 9, 9, 9):
        if x8_cuts[-1] >= n8:
            break
        x8_cuts.append(min(x8_cuts[-1] + step, n8))
    x8_chunks = list(zip(x8_cuts, x8_cuts[1:]))
    w8_cuts = [0]
    for step in (2, 5, 5, 5):
        if w8_cuts[-1] >= c:
            break
        w8_cuts.append(min(w8_cuts[-1] + step, c))
    w8_chunks = list(zip(w8_cuts, w8_cuts[1:]))

    with tile.TileContext(nc) as tc:
        with (
            tc.tile_pool(name="const", bufs=1) as cpool,
            tc.tile_pool(name="xpool", bufs=1) as xpool,
            tc.tile_pool(name="tpF", bufs=5) as tpF,
            tc.tile_pool(name="tpB", bufs=5) as tpB,
            tc.tile_pool(name="tp8", bufs=5) as tp8,
            tc.tile_pool(name="opool", bufs=5) as opool,
            tc.tile_pool(name="ps1", bufs=4, space="PSUM") as ps1,
            tc.tile_pool(name="ps2", bufs=4, space="PSUM") as ps2,
        ):
            # PE warmup bridge (vector engine boots fast): keeps the PE
            # busy while the first input transfers land (~5us)
            wu_sb = cpool.tile([128, 128], f16, name="wu")
            nc.vector.memset(wu_sb[:], 0.25)
            for _ in range(NWARM):
                wu_ps = ps2.tile([128, 2, N], f32, tag="t2")
                nc.tensor.matmul(
                    wu_ps[:, 0, 0:128], lhsT=wu_sb[:], rhs=wu_sb[:],
                    start=True, stop=True,
                )

            # loads, need-order, all on the sync HW-DGE ring (gpsimd
            # software-DGE transfers are several times slower)
            wF_sb = wM_sb = w8_sb = xF_sb = x8_sb = None
            if a:
                wF_sb = cpool.tile([128, a, 2, N], f16, name="wF")
            if nF:
                xF_sb = xpool.tile([128, nF, 2, N], f16, name="xF")
            if b:
                wM_sb = cpool.tile([128, b, 2, N], bf, name="wM")
            if n8:
                x8_sb = xpool.tile([128, n8, 2, N], f8, name="x8")
            if c:
                w8_sb = cpool.tile([128, c, 2, N], f8, name="w8")

            def load_x8(i):
                if i < len(x8_chunks):
                    s, e = x8_chunks[i]
                    nc.sync.dma_start(x8_sb[:, s:e], x8_d[:, s:e])

            def load_w8(i):
                if i < len(w8_chunks):
                    s, e = w8_chunks[i]
                    nc.sync.dma_start(w8_sb[:, s:e], w8_d[:, s:e])

            def load_xF(s, e):
                if nF and e > min(s, nF):
                    e = min(e, nF)
                    nc.sync.dma_start(xF_sb[:, s:e], xF_d[:, s:e])

            if a:
                nc.sync.dma_start(wF_sb[:], wF_d)
            load_xF(0, 2)
            load_w8(0)
            load_x8(0)
            load_xF(2, 4)
            load_x8(1)
            if b:
                nc.sync.dma_start(wM_sb[:], wM_d)
            load_w8(1)
            load_x8(2)
            load_xF(4, nF)
            load_w8(2)
            for i in range(3, len(x8_chunks)):
                load_x8(i)
            load_w8(3)

            def img_ctx(img):
                slot = img // 3
                cls = 0 if slot < a else (1 if slot < ab else 2)
                if cls == 0:
                    xt, wv = xF_sb[:, img], wF_sb[:, slot]
                elif cls == 1:
                    xt, wv = x8_sb[:, img - nF], wM_sb[:, slot - a]
                else:
                    xt, wv = x8_sb[:, img - nF], w8_sb[:, slot - ab]
                return slot, cls, xt, wv

            def copy(out_ap, ps_ap, cls, eng):
                if cls != 0:
                    b16 = ps_ap.bitcast(bf)
                    in_ap = b16[:, :, 1::2] if len(b16.shape) == 3 else b16[:, 1::2]
                else:
                    in_ap = ps_ap
                if eng == 0:
                    nc.vector.tensor_copy(out=out_ap, in_=in_ap)
                else:
                    nc.scalar.copy(out_ap, in_ap)

            t1sb = {}
            otiles = {}

            def stage_t1(img, pos):
                slot, cls, xt, wv = img_ctx(img)
                t1_ps = ps1.tile([128, 2, N], f32)
                for mb in range(2):
                    if cls == 2:
                        # fp8 DoubleRow: both k-tiles in one instruction at
                        # 2x MAC rate (accumulating fp8 pairs fall to 1x)
                        nc.tensor.matmul(
                            t1_ps[:, mb, :],
                            lhsT=xt[:, :, mb * 128:(mb + 1) * 128],
                            rhs=wv[:], start=True, stop=True,
                            perf_mode=mybir.MatmulPerfMode.DoubleRow,
                        )
                        continue
                    for aa in range(2):
                        nc.tensor.matmul(
                            t1_ps[:, mb, :],
                            lhsT=xt[:, aa, mb * 128:(mb + 1) * 128],
                            rhs=wv[:, aa],
                            start=(aa == 0),
                            stop=(aa == 1),
                        )
                if cls == 0:
                    t1_sb = tpF.tile([128, 2, N], f16)
                elif cls == 1:
                    t1_sb = tpB.tile([128, 2, N], bf)
                else:
                    t1_sb = tp8.tile([128, 2, N], f8)
                copy(t1_sb[:], t1_ps[:], cls, eng=pos % 2)
                t1sb[img] = t1_sb

            def stage_t2(img, pos):
                slot, cls, xt, wv = img_ctx(img)
                t1_sb = t1sb.pop(img)
                t2_ps = ps2.tile([128, 2, N], f32, tag="t2")
                for mb in range(2):
                    if cls == 2:
                        nc.tensor.matmul(
                            t2_ps[:, mb, :],
                            lhsT=t1_sb[:, :, mb * 128:(mb + 1) * 128],
                            rhs=wv[:], start=True, stop=True,
                            perf_mode=mybir.MatmulPerfMode.DoubleRow,
                        )
                        continue
                    for aa in range(2):
                        nc.tensor.matmul(
                            t2_ps[:, mb, :],
                            lhsT=t1_sb[:, aa, mb * 128:(mb + 1) * 128],
                            rhs=wv[:, aa],
                            start=(aa == 0),
                            stop=(aa == 1),
                        )
                j = img % 3
                if j == 0:
                    otiles[slot] = opool.tile(
                        [128, 3, 2, N], f16 if cls == 0 else bf,
                        tag=f"o{min(cls, 1)}", name=f"ot{min(cls, 1)}",
                    )
                ot = otiles[slot]
                last = pos >= IMGS - 4
                if last:
                    copy(ot[:, j, 0], t2_ps[:, 0], cls, eng=0)
                    copy(ot[:, j, 1], t2_ps[:, 1], cls, eng=1)
                else:
                    copy(ot[:, j], t2_ps[:], cls, eng=(pos + 1) % 2)

                od = oF_d if cls == 0 else o8_d
                base = 0 if cls == 0 else nF
                odi = od[:, 3 * slot + j - base]
                # tail-mode slot: its last image lands in the final positions,
                # so store per-image to keep the drain-out overlapped
                tail = pos_of[3 * slot + 2] >= IMGS - 6
                if last or tail:
                    # alternate HW-DGE queues so tail dispatches overlap
                    q = nc.sync if pos % 2 == 0 else nc.scalar
                    q.dma_start(odi, ot[:, j])
                elif j == 2:
                    nc.sync.dma_start(
                        od[:, 3 * slot - base:3 * slot + 3 - base], ot[:])

            # depth-3 software pipeline over the interleaved order: t2 of
            # position p sits after t1 of position p+3, giving every drain
            # a deep backlog so semaphore latency never idles the engines
            D = 3
            for p in range(IMGS):
                stage_t1(seq[p], p)
                if p >= D:
                    stage_t2(seq[p - D], p - D)
            for p in range(IMGS - D, IMGS):
                stage_t2(seq[p], p)

    nc.compile()
    return nc


def _get_nc(a, b, c):
    key = (a, b, c)
    if key not in _NC_CACHE:
        _NC_CACHE[key] = _build_nc(a, b, c)
    return _NC_CACHE[key]


def _host_w(blur_sigmas, fwd_steps):
    """Per-batch W_b = (D diag(e_b) D)^T, float64, [B, N, N]."""
    sig = np.asarray(blur_sigmas, dtype=np.float64)
    steps = np.asarray(fwd_steps).astype(np.int64)
    n = np.arange(N, dtype=np.float64)
    D = np.sqrt(2.0 / N) * np.cos(np.pi * (n[None, :] + 0.5) * n[:, None] / N)
    D[0] *= 1.0 / np.sqrt(2.0)
    freqs = np.pi * n / N
    uniq, inv = np.unique(steps, return_inverse=True)
    ms = np.empty((len(uniq), N, N), dtype=np.float64)
    for i, s in enumerate(uniq):
        t = sig[s] ** 2 / 2.0
        e = np.exp(-(freqs ** 2) * t)
        ms[i] = (D @ (e[:, None] * D)).T
    return ms[inv]


def _classify(steps):
    """class 0: k<=1 (f16), 1: 2<=k<=5 (mixed), 2: k>=6 (f8)."""
    k = np.asarray(steps).astype(np.int64)
    return np.where(k <= 1, 0, np.where(k <= 5, 1, 2))


def kernel(x, blur_sigmas, fwd_steps):
    global LAST_EXEC_TIME_NS
    from concourse import bass_utils

    x = np.asarray(x)
    assert x.shape == (BATCH, CHANNELS, N, N), x.shape
    steps = np.asarray(fwd_steps).astype(np.int64)
    cls = _classify(steps)
    n0 = int((cls == 0).sum())
    n1 = int((cls == 1).sum())
    a = min(-(-n0 // N_CORES), PB)
    ab = min(max(a, -(-(n0 + n1) // N_CORES)), PB)
    b = ab - a
    c = PB - ab

    # batch order: sort by (class, k); slot s of core i is batch order[8s+i]
    order = np.lexsort((steps, cls))
    w_all = _host_w(blur_sigmas, fwd_steps)

    # device image layout [128, 2, N]: [p, aa, w] = X[aa*128+p, w]
    xr = x.astype(np.float32).reshape(BATCH, CHANNELS, 2, 128, N)

    in_maps = []
    for core in range(N_CORES):
        bs = order[core::N_CORES]
        assert len(bs) == PB
        m = {}
        if a:
            xf = xr[bs[:a]].transpose(3, 0, 1, 2, 4).reshape(128, 3 * a, 2 * N)
            m["xF"] = np.ascontiguousarray(xf.astype(np.float16))
        if b + c:
            x8 = xr[bs[a:]].transpose(3, 0, 1, 2, 4).reshape(128, 3 * (b + c), 2 * N)
            m["x8"] = np.ascontiguousarray(x8.astype(F8))
        # W layout [128, slot, aa, h] = W_b[aa*128+p, h]
        wb = w_all[bs].reshape(PB, 2, 128, N).transpose(2, 0, 1, 3)
        if a:
            m["wF"] = np.ascontiguousarray(wb[:, :a].astype(np.float16))
        if b:
            m["wM"] = np.ascontiguousarray(wb[:, a:ab].astype(BF16))
        if c:
            m["w8"] = np.ascontiguousarray(wb[:, ab:].astype(F8))
        in_maps.append(m)

    nc = _get_nc(a, b, c)
    trace = os.environ.get("BASS_DCT_TRACE", "0") == "1"
    if trace:
        _install_ntff_hook()

    res = None
    exec_ns = None
    # first execution of a freshly loaded NEFF is occasionally slow (cold
    # device caches) or raises a transient NRT error; run once untraced to
    # warm up, then measure the steady-state execution with trace
    import tempfile
    for attempt in range(4):
        try:
            kwargs = {}
            if trace and attempt > 0:
                kwargs["trace"] = True
                base = os.environ.get("BASS_DCT_TRACE_DIR")
                if base:
                    td = os.path.join(base, f"attempt{attempt}")
                    os.makedirs(td, exist_ok=True)
                else:
                    td = tempfile.mkdtemp(prefix="dct_trace_")
                kwargs["tmpdir"] = td
            r = bass_utils.run_bass_kernel_spmd(
                nc, in_maps, core_ids=list(range(N_CORES)), **kwargs
            )
            res = r
            if r.exec_time_ns is not None:
                exec_ns = r.exec_time_ns
            if attempt >= 1:
                break
        except Exception:
            if attempt == 3 and res is None:
                raise
            import time as _time

            _time.sleep(2.0)
    LAST_EXEC_TIME_NS = exec_ns

    # gather: [img, p, (aa, w)] -> out[batch, ch, aa*128+p, w]
    out = np.empty((BATCH, CHANNELS, N, N), dtype=np.float32)
    for core in range(N_CORES):
        bs = order[core::N_CORES]
        parts = []
        if a:
            ocF = np.asarray(res.results[core]["oF"], dtype=np.float32)
            parts.append(ocF.reshape(128, 3 * a, 2, N))
        if b + c:
            oc8 = np.asarray(res.results[core]["o8"]).astype(np.float32)
            parts.append(oc8.reshape(128, 3 * (b + c), 2, N))
        oc = np.concatenate(parts, axis=1) if len(parts) > 1 else parts[0]
        # [p, img, aa, w] -> [img, aa, p, w]
        oc = oc.transpose(1, 2, 0, 3).reshape(PB, CHANNELS, N, N)
        out[bs] = oc
    return np.ascontiguousarray(out)


# revision 39
# speedup vs baseline: 1.1738x; 1.0294x over previous
"""DCT heat-blur kernel for Trainium2 (8 NeuronCores, Bass/Tile).

Math (matches reference einsums): per image X,
    out = (D diag(e) D) X (D diag(e) D)^T = W^T X W,  W = (D diag(e) D)^T,
with e[k] = exp(-(pi k/N)^2 sigma_b^2/2).  W_b is built on host per batch.
Device: t1 = X^T W, out = t1^T W  (two GEMM passes per image).

Hybrid precision (validated by exact host emulation of the quantization
on the real inputs; harness tolerance is 2e-2 max-rel, achieved ~1e-2):
  class 0 (k<=1):    X f16,  W f16,  f16 drains            (err ~8e-4)
  class 1 (2<=k<=5): X f8e4, W bf16, bf16-punned drains    (err <=1.1e-2)
  class 2 (k>=6):    X f8e4, W f8e4, t1 f8, bf16-pun out   (err <=9.7e-3)
Class-2 GEMMs use fp8 DoubleRow matmuls: both K=128 k-tiles of the K=256
contraction in ONE instruction at 2x MAC rate (~107ns per [128x256]
output vs 2x107 for fp16 accumulation pairs; fp8 with PSUM-accumulation
pairs falls back to 1x, so DoubleRow is required for the 2x).  fp8 X/W
also halve input DMA.  Blur washes out quantization noise, so larger
sigma tolerates coarser inputs.

PSUM drains (psum f32 -> sbuf) are the kernel bottleneck: only the
vector and scalar engines can read PSUM (GPSIMD cannot, DMA cannot),
at ~1 element/lane/cycle regardless of dtype => ~590-690ns per
[128, 512-elem] drain.  Two drains per image (t1, out) over two engines
=> ~590ns/image floor.  The "bf16 pun" (bitcast the f32 psum to bf16
pairs, copy the high halves = round-toward-zero bf16) keeps the drains
single-pass while producing 16-bit operands for the second GEMM and the
output store.

Schedule per core (48 images):
 - classes INTERLEAVED (a class0/1 image every ~4 positions, first 4
   positions class0): f16 images are PE-bound (872ns) while f8 images
   are drain-bound (~600ns), so interleaving overlaps their idle times.
 - depth-3 software pipeline: t2 of position p is emitted after t1 of
   position p+3, so every drain has a deep backlog and semaphore
   latency never idles the copy engines.
 - all transfers on the sync HW-DGE ring (gpsimd software-DGE transfers
   are several times slower); partition-major DRAM layouts give >=1KB
   contiguous runs per descriptor; loads dispatched in need-order.
 - a short vector-memset + dummy-matmul bridge covers engine boot and
   the ~4us DGE ring spin-up before the first input lands.
 - per-slot grouped stores; per-image near the end to shorten the tail.

Sharding: pure data parallel over batches; batches sorted by (class, k)
and dealt 8-way so every core gets identical per-class slot counts
(a,b,c) -- required because all cores run one SPMD NEFF.  The NEFF is
compiled on first call for the (a,b,c) derived from the actual
fwd_steps (cached per (a,b,c)).
"""

import os
import numpy as np
import ml_dtypes

BATCH = 128
CHANNELS = 3
N = 256
N_CORES = 8
PB = BATCH // N_CORES          # batch slots per core
IMGS = PB * CHANNELS           # images per core
NWARM = 12

F8 = ml_dtypes.float8_e4m3
BF16 = ml_dtypes.bfloat16

LAST_EXEC_TIME_NS = None
_NC_CACHE = {}


def _install_ntff_hook():
    """Wire antenv.axon_hooks (missing in this image) so trace=True works."""
    import sys
    import types

    if "antenv.axon_hooks" in sys.modules:
        return
    try:
        import trn_agent_boot.trn_boot as tb

        hook = tb._ntff_profile_via_ctypes("/opt/axon/libaxon_pjrt.so")
    except Exception:
        hook = None
    m = types.ModuleType("antenv.axon_hooks")
    m.get_axon_ntff_profile_hook = lambda: hook
    m.set_axon_ntff_profile_hook = lambda h: None
    sys.modules["antenv.axon_hooks"] = m


def _build_nc(a, b, c):
    import concourse.bacc as bacc
    import concourse.tile as tile
    import concourse.mybir as mybir

    f32 = mybir.dt.float32
    f16 = mybir.dt.float16
    bf = mybir.dt.bfloat16
    f8 = mybir.dt.float8e4

    nF = 3 * a                  # f16 images (class 0)
    n8 = 3 * (b + c)            # f8 images (classes 1, 2)
    ab = a + b

    nc = bacc.Bacc("TRN2", target_bir_lowering=False, debug=False)
    xF_d = x8_d = wF_d = wM_d = w8_d = oF_d = o8_d = None
    if nF:
        xF_d = nc.dram_tensor("xF", [128, nF, 2, N], f16, kind="ExternalInput").ap()
        oF_d = nc.dram_tensor("oF", [128, nF, 2, N], f16, kind="ExternalOutput").ap()
        wF_d = nc.dram_tensor("wF", [128, a, 2, N], f16, kind="ExternalInput").ap()
    if n8:
        x8_d = nc.dram_tensor("x8", [128, n8, 2, N], f8, kind="ExternalInput").ap()
        o8_d = nc.dram_tensor("o8", [128, n8, 2, N], bf, kind="ExternalOutput").ap()
    if b:
        wM_d = nc.dram_tensor("wM", [128, b, 2, N], bf, kind="ExternalInput").ap()
    if c:
        w8_d = nc.dram_tensor("w8", [128, c, 2, N], f8, kind="ExternalInput").ap()

    # interleave classes: a class0/1 image every ~IMGS/n01 positions so the
    # PE-bound f16 images and drain-bound f8 images overlap their idle times
    n01 = 3 * ab
    head = min(4, n01)
    seq = list(range(head))
    i01, i2, err = head, 0, 0
    rem01 = n01 - head
    rem = IMGS - head
    for p in range(rem):
        err += rem01
        if err >= rem and i01 < n01:
            err -= rem
            seq.append(i01)
            i01 += 1
        elif i2 < IMGS - n01:
            seq.append(n01 + i2)
            i2 += 1
        else:
            seq.append(i01)
            i01 += 1
    pos_of = {img: p for p, img in enumerate(seq)}

    # x8 load chunks (front-loaded small), w8 chunks
    x8_cuts = [0]
    for step in (3, 6, 9, 9, 9, 9, 9):
        if x8_cuts[-1] >= n8:
            break
        x8_cuts.append(min(x8_cuts[-1] + step, n8))
    x8_chunks = list(zip(x8_cuts, x8_cuts[1:]))
    w8_cuts = [0]
    for step in (2, 5, 5, 5):
        if w8_cuts[-1] >= c:
            break
        w8_cuts.append(min(w8_cuts[-1] + step, c))
    w8_chunks = list(zip(w8_cuts, w8_cuts[1:]))

    with tile.TileContext(nc) as tc:
        with (
            tc.tile_pool(name="const", bufs=1) as cpool,
            tc.tile_pool(name="xpool", bufs=1) as xpool,
            tc.tile_pool(name="tpF", bufs=5) as tpF,
            tc.tile_pool(name="tpB", bufs=5) as tpB,
            tc.tile_pool(name="tp8", bufs=5) as tp8,
            tc.tile_pool(name="opool", bufs=5) as opool,
            tc.tile_pool(name="ps1", bufs=4, space="PSUM") as ps1,
            tc.tile_pool(name="ps2", bufs=4, space="PSUM") as ps2,
        ):
            # PE warmup bridge (vector engine boots fast): keeps the PE
            # busy while the first input transfers land (~5us)
            wu_sb = cpool.tile([128, 128], f16, name="wu")
            nc.vector.memset(wu_sb[:], 0.25)
            for _ in range(NWARM):
                wu_ps = ps2.tile([128, 2, N], f32, tag="t2")
                nc.tensor.matmul(
                    wu_ps[:, 0, 0:128], lhsT=wu_sb[:], rhs=wu_sb[:],
                    start=True, stop=True,
                )

            # loads, need-order, all on the sync HW-DGE ring (gpsimd
            # software-DGE transfers are several times slower)
            wF_sb = wM_sb = w8_sb = xF_sb = x8_sb = None
            if a:
                wF_sb = cpool.tile([128, a, 2, N], f16, name="wF")
            if nF:
                xF_sb = xpool.tile([128, nF, 2, N], f16, name="xF")
            if b:
                wM_sb = cpool.tile([128, b, 2, N], bf, name="wM")
            if n8:
                x8_sb = xpool.tile([128, n8, 2, N], f8, name="x8")
            if c:
                w8_sb = cpool.tile([128, c, 2, N], f8, name="w8")

            def load_x8(i):
                if i < len(x8_chunks):
                    s, e = x8_chunks[i]
                    nc.sync.dma_start(x8_sb[:, s:e], x8_d[:, s:e])

            def load_w8(i):
                if i < len(w8_chunks):
                    s, e = w8_chunks[i]
                    nc.sync.dma_start(w8_sb[:, s:e], w8_d[:, s:e])

            def load_xF(s, e):
                if nF and e > min(s, nF):
                    e = min(e, nF)
                    nc.sync.dma_start(xF_sb[:, s:e], xF_d[:, s:e])

            if a:
                nc.sync.dma_start(wF_sb[:], wF_d)
            load_xF(0, 2)
            load_w8(0)
            load_x8(0)
            load_xF(2, 4)
            load_x8(1)
            if b:
                nc.sync.dma_start(wM_sb[:], wM_d)
            load_w8(1)
            load_x8(2)
            load_xF(4, nF)
            load_w8(2)
            for i in range(3, len(x8_chunks)):
                load_x8(i)
            load_w8(3)

            def img_ctx(img):
                slot = img // 3
                cls = 0 if slot < a else (1 if slot < ab else 2)
                if cls == 0:
                    xt, wv = xF_sb[:, img], wF_sb[:, slot]
                elif cls == 1:
                    xt, wv = x8_sb[:, img - nF], wM_sb[:, slot - a]
                else:
                    xt, wv = x8_sb[:, img - nF], w8_sb[:, slot - ab]
                return slot, cls, xt, wv

            def copy(out_ap, ps_ap, cls, eng):
                if cls != 0:
                    b16 = ps_ap.bitcast(bf)
                    in_ap = b16[:, :, 1::2] if len(b16.shape) == 3 else b16[:, 1::2]
                else:
                    in_ap = ps_ap
                if eng == 0:
                    nc.vector.tensor_copy(out=out_ap, in_=in_ap)
                else:
                    nc.scalar.copy(out_ap, in_ap)

            t1sb = {}
            otiles = {}

            def stage_t1(img, pos):
                slot, cls, xt, wv = img_ctx(img)
                t1_ps = ps1.tile([128, 2, N], f32)
                for mb in range(2):
                    if cls == 2:
                        # fp8 DoubleRow: both k-tiles in one instruction at
                        # 2x MAC rate (accumulating fp8 pairs fall to 1x)
                        nc.tensor.matmul(
                            t1_ps[:, mb, :],
                            lhsT=xt[:, :, mb * 128:(mb + 1) * 128],
                            rhs=wv[:], start=True, stop=True,
                            perf_mode=mybir.MatmulPerfMode.DoubleRow,
                        )
                        continue
                    for aa in range(2):
                        nc.tensor.matmul(
                            t1_ps[:, mb, :],
                            lhsT=xt[:, aa, mb * 128:(mb + 1) * 128],
                            rhs=wv[:, aa],
                            start=(aa == 0),
                            stop=(aa == 1),
                        )
                if cls == 0:
                    t1_sb = tpF.tile([128, 2, N], f16)
                elif cls == 1:
                    t1_sb = tpB.tile([128, 2, N], bf)
                else:
                    t1_sb = tp8.tile([128, 2, N], f8)
                copy(t1_sb[:], t1_ps[:], cls, eng=pos % 2)
                t1sb[img] = t1_sb

            def stage_t2(img, pos):
                slot, cls, xt, wv = img_ctx(img)
                t1_sb = t1sb.pop(img)
                t2_ps = ps2.tile([128, 2, N], f32, tag="t2")
                for mb in range(2):
                    if cls == 2:
                        nc.tensor.matmul(
                            t2_ps[:, mb, :],
                            lhsT=t1_sb[:, :, mb * 128:(mb + 1) * 128],
                            rhs=wv[:], start=True, stop=True,
                            perf_mode=mybir.MatmulPerfMode.DoubleRow,
                        )
                        continue
                    for aa in range(2):
                        nc.tensor.matmul(
                            t2_ps[:, mb, :],
                            lhsT=t1_sb[:, aa, mb * 128:(mb + 1) * 128],
                            rhs=wv[:, aa],
                            start=(aa == 0),
                            stop=(aa == 1),
                        )
                j = img % 3
                if j == 0:
                    otiles[slot] = opool.tile(
                        [128, 3, 2, N], f16 if cls == 0 else bf,
                        tag=f"o{min(cls, 1)}", name=f"ot{min(cls, 1)}",
                    )
                ot = otiles[slot]
                last = pos >= IMGS - 4
                if last:
                    copy(ot[:, j, 0], t2_ps[:, 0], cls, eng=0)
                    copy(ot[:, j, 1], t2_ps[:, 1], cls, eng=1)
                else:
                    copy(ot[:, j], t2_ps[:], cls, eng=(pos + 1) % 2)

                od = oF_d if cls == 0 else o8_d
                base = 0 if cls == 0 else nF
                odi = od[:, 3 * slot + j - base]
                # tail-mode slot: its last image lands in the final positions,
                # so store per-image to keep the drain-out overlapped
                tail = pos_of[3 * slot + 2] >= IMGS - 6
                if last or tail:
                    # alternate HW-DGE queues so tail dispatches overlap
                    q = nc.sync if pos % 2 == 0 else nc.scalar
                    q.dma_start(odi, ot[:, j])
                elif j == 2:
                    nc.sync.dma_start(
                        od[:, 3 * slot - base:3 * slot + 3 - base], ot[:])

            # depth-3 software pipeline over the interleaved order: t2 of
            # position p sits after t1 of position p+3, giving every drain
            # a deep backlog so semaphore latency never idles the engines
            D = 3
            for p in range(IMGS):
                stage_t1(seq[p], p)
                if p >= D:
                    stage_t2(seq[p - D], p - D)
            for p in range(IMGS - D, IMGS):
                stage_t2(seq[p], p)

    nc.compile()
    return nc


def _get_nc(a, b, c):
    key = (a, b, c)
    if key not in _NC_CACHE:
        _NC_CACHE[key] = _build_nc(a, b, c)
    return _NC_CACHE[key]


def _host_w(blur_sigmas, fwd_steps):
    """Per-batch W_b = (D diag(e_b) D)^T, float64, [B, N, N]."""
    sig = np.asarray(blur_sigmas, dtype=np.float64)
    steps = np.asarray(fwd_steps).astype(np.int64)
    n = np.arange(N, dtype=np.float64)
    D = np.sqrt(2.0 / N) * np.cos(np.pi * (n[None, :] + 0.5) * n[:, None] / N)
    D[0] *= 1.0 / np.sqrt(2.0)
    freqs = np.pi * n / N
    uniq, inv = np.unique(steps, return_inverse=True)
    ms = np.empty((len(uniq), N, N), dtype=np.float64)
    for i, s in enumerate(uniq):
        t = sig[s] ** 2 / 2.0
        e = np.exp(-(freqs ** 2) * t)
        ms[i] = (D @ (e[:, None] * D)).T
    return ms[inv]


def _classify(steps):
    """class 0: k<=1 (f16), 1: 2<=k<=5 (mixed), 2: k>=6 (f8)."""
    k = np.asarray(steps).astype(np.int64)
    return np.where(k <= 1, 0, np.where(k <= 5, 1, 2))


def kernel(x, blur_sigmas, fwd_steps):
    global LAST_EXEC_TIME_NS
    from concourse import bass_utils

    x = np.asarray(x)
    assert x.shape == (BATCH, CHANNELS, N, N), x.shape
    steps = np.asarray(fwd_steps).astype(np.int64)
    cls = _classify(steps)
    n0 = int((cls == 0).sum())
    n1 = int((cls == 1).sum())
    a = min(-(-n0 // N_CORES), PB)
    ab = min(max(a, -(-(n0 + n1) // N_CORES)), PB)
    b = ab - a
    c = PB - ab

    # batch order: sort by (class, k); slot s of core i is batch order[8s+i]
    order = np.lexsort((steps, cls))
    w_all = _host_w(blur_sigmas, fwd_steps)

    # device image layout [128, 2, N]: [p, aa, w] = X[aa*128+p, w]
    xr = x.astype(np.float32).reshape(BATCH, CHANNELS, 2, 128, N)

    in_maps = []
    for core in range(N_CORES):
        bs = order[core::N_CORES]
        assert len(bs) == PB
        m = {}
        if a:
            xf = xr[bs[:a]].transpose(3, 0, 1, 2, 4).reshape(128, 3 * a, 2 * N)
            m["xF"] = np.ascontiguousarray(xf.astype(np.float16))
        if b + c:
            x8 = xr[bs[a:]].transpose(3, 0, 1, 2, 4).reshape(128, 3 * (b + c), 2 * N)
            m["x8"] = np.ascontiguousarray(x8.astype(F8))
        # W layout [128, slot, aa, h] = W_b[aa*128+p, h]
        wb = w_all[bs].reshape(PB, 2, 128, N).transpose(2, 0, 1, 3)
        if a:
            m["wF"] = np.ascontiguousarray(wb[:, :a].astype(np.float16))
        if b:
            m["wM"] = np.ascontiguousarray(wb[:, a:ab].astype(BF16))
        if c:
            m["w8"] = np.ascontiguousarray(wb[:, ab:].astype(F8))
        in_maps.append(m)

    nc = _get_nc(a, b, c)
    trace = os.environ.get("BASS_DCT_TRACE", "0") == "1"
    if trace:
        _install_ntff_hook()

    res = None
    exec_ns = None
    # first execution of a freshly loaded NEFF is occasionally slow (cold
    # device caches) or raises a transient NRT error; run once untraced to
    # warm up, then measure the steady-state execution with trace
    import tempfile
    for attempt in range(4):
        try:
            kwargs = {}
            if trace and attempt > 0:
                kwargs["trace"] = True
                base = os.environ.get("BASS_DCT_TRACE_DIR")
                if base:
                    td = os.path.join(base, f"attempt{attempt}")
                    os.makedirs(td, exist_ok=True)
                else:
                    td = tempfile.mkdtemp(prefix="dct_trace_")
                kwargs["tmpdir"] = td
            r = bass_utils.run_bass_kernel_spmd(
                nc, in_maps, core_ids=list(range(N_CORES)), **kwargs
            )
            res = r
            if r.exec_time_ns is not None:
                exec_ns = r.exec_time_ns
            if attempt >= 1:
                break
        except Exception:
            if attempt == 3 and res is None:
                raise
            import time as _time

            _time.sleep(2.0)
    LAST_EXEC_TIME_NS = exec_ns

    # gather: [img, p, (aa, w)] -> out[batch, ch, aa*128+p, w]
    out = np.empty((BATCH, CHANNELS, N, N), dtype=np.float32)
    for core in range(N_CORES):
        bs = order[core::N_CORES]
        parts = []
        if a:
            ocF = np.asarray(res.results[core]["oF"], dtype=np.float32)
            parts.append(ocF.reshape(128, 3 * a, 2, N))
        if b + c:
            oc8 = np.asarray(res.results[core]["o8"]).astype(np.float32)
            parts.append(oc8.reshape(128, 3 * (b + c), 2, N))
        oc = np.concatenate(parts, axis=1) if len(parts) > 1 else parts[0]
        # [p, img, aa, w] -> [img, aa, p, w]
        oc = oc.transpose(1, 2, 0, 3).reshape(PB, CHANNELS, N, N)
        out[bs] = oc
    return np.ascontiguousarray(out)
